# revision 27
# baseline (speedup 1.0000x reference)
"""Trainium2 Bass kernel for nn_DeformNet (dense per-point MLP network).

Strategy: pure data parallel over batch (bs=8 -> 1 batch item per NeuronCore).
All matmuls run in "channels on partitions" layout: activations are [C, n_pts]
tiles, weights are pre-transposed [Cin, Cout] (lhsT), so out = W @ x lands as
[Cout, n_pts] in PSUM. The per-sample category selection (cat_id) is applied
host-side by slicing the final assignment/deformation weight rows before
shipping them to the device (each core only computes the 1024 assignment
channels + 3 deformation channels its batch item actually needs). The big
[nv, n_pts] result is produced transposed on device and swapped on the host.

Matmuls use the float32r PE mode (TF32-like rounding, ~234 ns/matmul at
N=512 when warm). The instance-geometry and category-local 3-layer chains
are fused pairwise with block-diagonal weights so one matmul computes both.
PSUM drains are split between ScalarE and VectorE; a few fp32 dummy matmuls
at kernel start pre-warm the PE clock (HAM) before the real stream begins.
"""

import os
import sys

for _p in ("/opt/trn_rl_repo", "/root/.axon_site/_ro/trn_rl_repo"):
    if _p not in sys.path and os.path.isdir(_p):
        sys.path.append(_p)

import numpy as np

import concourse.bass as bass
import concourse.tile as tile
from concourse import bacc, mybir
from concourse.bass_utils import run_bass_kernel_spmd

F32 = mybir.dt.float32
F32R = mybir.dt.float32r
RELU = mybir.ActivationFunctionType.Relu
COPY = mybir.ActivationFunctionType.Identity
ADD = mybir.AluOpType.add
MAX = mybir.AluOpType.max

NPTS = 1024
NV = 1024
NCAT = 6
NHALF = 512  # fp32 moving-operand max per matmul

WSM_W = 744  # 704 weight cols + 37 bias cols (padded)
BIAS_BASE = 704

# bias column map (columns BIAS_BASE+c of the wsm tile)
BIAS_COLS = {
    "p1": 0, "p2": 1, "p3": 2,   # paired ig/cl biases: rows 0:64 ig, 64:128 cl
    "ic": 3, "cg": 4, "iglob": 5,
    "t64a": 6,    # 4 cols
    "t64b": 10,
    "t128a": 11,  # 4 cols
    "t128b": 15,
    "as0": 16,    # 4 cols
    "as1": 20,    # 2 cols
    "as2": 22,    # 8 cols
    "de0": 30,    # 4 cols
    "de1": 34,    # 2 cols
    "de2": 36,
}

_PROGRAM = None


def _build_program():
    nc = bacc.Bacc("TRN2", target_bir_lowering=False, debug=False, num_devices=8)

    dram_in = {}
    for name, shape in [
        ("xin", [38, NPTS]),
        ("wsm", [128, WSM_W]),
        ("wt64a", [128, 512]), ("wt64b", [128, 256]),
        ("wt128a", [128, 1024]), ("wt128b", [128, 512]),
        ("was0", [128, 512]), ("was1", [128, 1024]), ("was2", [128, 2048]),
        ("wde0", [128, 512]), ("wde1", [128, 1024]), ("wde2", [128, 6]),
    ]:
        dram_in[name] = nc.dram_tensor(name, shape, F32, kind="ExternalInput")

    out_a = nc.dram_tensor("assign_T", [NV, NPTS], F32, kind="ExternalOutput")
    out_d = nc.dram_tensor("deltas_T", [3, NPTS], F32, kind="ExternalOutput")

    with tile.TileContext(nc) as tc:
        with (
            tc.tile_pool(name="w", bufs=1) as wp,
            tc.tile_pool(name="a", bufs=1) as ap_,
            tc.tile_pool(name="hb", bufs=2) as hp,
            tc.tile_pool(name="ps", bufs=4, space="PSUM") as pp,
            tc.tile_pool(name="o", bufs=4) as op_,
        ):
            # ---- critical-path loads first on sync; big weights spread
            # across the other DMA-capable sequencers ----
            wsm_t = wp.tile([128, WSM_W], F32R, tag="wsm")
            nc.sync.dma_start(wsm_t[:], dram_in["wsm"][:].bitcast(F32R))
            xpp = wp.tile([6, NPTS], F32R, tag="xpp")
            xemb = wp.tile([32, NPTS], F32R, tag="xemb")
            nc.sync.dma_start(xpp[:], dram_in["xin"][0:6, :].bitcast(F32R))
            nc.sync.dma_start(xemb[:], dram_in["xin"][6:38, :].bitcast(F32R))

            # ---- PE warm-up: fp32 dummy matmuls (4 cyc/row -> dense PE
            # busy) prime the HAM activity window so the real matmul stream
            # runs at 2.4 GHz ----
            wz = wp.tile([128, NHALF], F32, tag="wz")
            nc.vector.memset(wz[:], 0.0)
            for i in range(5):
                psw = pp.tile([128, NHALF], F32, tag="ps")
                nc.tensor.matmul(psw[:], wz[:, 0:128], wz[:],
                                 start=True, stop=True)

            DMA_ENG = {
                "wt64a": nc.scalar, "wt64b": nc.scalar,
                "wt128a": nc.scalar, "wt128b": nc.scalar,
                "was0": nc.gpsimd, "wde0": nc.gpsimd,
                "was1": nc.gpsimd, "wde1": nc.gpsimd,
                "was2": nc.sync, "wde2": nc.sync,
            }
            W = {"wsm": wsm_t}
            for name in ("wt64a", "wt64b", "wt128a", "wt128b",
                         "was0", "wde0", "was1", "wde1", "was2", "wde2"):
                cols = dram_in[name].shape[1]
                t = wp.tile([128, cols], F32R, tag=name)
                DMA_ENG[name].dma_start(t[:], dram_in[name][:].bitcast(F32R))
                W[name] = t

            wsmb = wsm_t[:].bitcast(F32)

            def bias(key, rows, off=0, prow=0):
                c = BIAS_BASE + BIAS_COLS[key] + off
                return wsmb[prow:prow + rows, c:c + 1]

            def mm(ps, lhsT, rhs, start, stop):
                nc.tensor.matmul(ps, lhsT, rhs, start=start, stop=stop)

            def drain_act(ps, dst, b, relu=True):
                nc.scalar.activation(dst, ps, RELU if relu else COPY, bias=b)

            def drain_dve_relu(ps, dst, b):
                nc.vector.tensor_scalar(dst, ps, b, 0.0, ADD, MAX)

            def drain_dve_bias(ps, dst, b):
                nc.vector.tensor_scalar(dst, ps, b, None, ADD)

            def drain2(ps, dst, b, relu=True):
                # split the drain: ACT takes the first half, DVE the second,
                # so PSUM evacuation latency halves and both engines load
                h = NHALF
                drain_act(ps[:, 0:h], dst[:, 0:h], b, relu)
                if relu:
                    drain_dve_relu(ps[:, h:2 * h], dst[:, h:2 * h], b)
                else:
                    drain_dve_bias(ps[:, h:2 * h], dst[:, h:2 * h], b)

            # ---- fused instance-geometry (rows 0:64) + category-local
            # (rows 64:128) chains via block-diagonal weights.  This serial
            # chain is latency-bound, so it runs at 256-wide chunks with
            # the drains pipelined behind the matmuls ----
            ab1 = ap_.tile([128, NPTS], F32R, tag="ab1")
            ab2 = ap_.tile([128, NPTS], F32R, tag="ab2")
            bcl = ap_.tile([128, NPTS], F32R, tag="bcl")  # cl3 out in rows 64:128
            tpe = ap_.tile([128, NPTS], F32R, tag="tpe")
            ig = ap_.tile([128, NPTS], F32R, tag="ig")
            cg = ap_.tile([128, NPTS], F32R, tag="cg")

            NQ = 256

            # pair1: K=6 rows (pts rows 0:3, prior rows 3:6)
            ps = pp.tile([128, NPTS], F32, tag="ps")
            for c in range(4):
                sl = slice(c * NQ, (c + 1) * NQ)
                mm(ps[:, sl], wsm_t[0:6, 0:128], xpp[0:6, sl],
                   start=True, stop=True)
                if c % 2 == 0:
                    drain_act(ps[:, sl], ab1[:, sl], bias("p1", 128))
                else:
                    drain_dve_relu(ps[:, sl], ab1[:, sl], bias("p1", 128))

            # ic is independent -> fills the PE while ab1 drains; its drain
            # is a partition-shifted write (psum rows 0:64 -> tpe rows
            # 64:128), which only ACT handles
            ps_ic = pp.tile([64, NPTS], F32, tag="ps")
            for n in range(2):
                sl = slice(n * NHALF, (n + 1) * NHALF)
                mm(ps_ic[:, sl], wsm_t[0:32, 384:448], xemb[:, sl],
                   start=True, stop=True)
                drain_act(ps_ic[:, sl], tpe[64:128, sl], bias("ic", 64))

            # pair2
            ps = pp.tile([128, NPTS], F32, tag="ps")
            for c in range(4):
                sl = slice(c * NQ, (c + 1) * NQ)
                mm(ps[:, sl], wsm_t[0:128, 128:256], ab1[:, sl],
                   start=True, stop=True)
                if c % 2 == 0:
                    drain_act(ps[:, sl], ab2[:, sl], bias("p2", 128))
                else:
                    drain_dve_relu(ps[:, sl], ab2[:, sl], bias("p2", 128))

            # pair3: rows 0:64 -> tpe (pts path), rows 64:128 -> bcl (cl3)
            ps = pp.tile([128, NPTS], F32, tag="ps")
            for c in range(4):
                sl = slice(c * NQ, (c + 1) * NQ)
                mm(ps[:, sl], wsm_t[0:128, 256:384], ab2[:, sl],
                   start=True, stop=True)
                drain_act(ps[0:64, sl], tpe[0:64, sl], bias("p3", 64))
                drain_dve_relu(ps[64:128, sl], bcl[64:128, sl],
                               bias("p3", 64, prow=64))

            # cg: weights placed at wsm rows 64:128 so lhsT/rhs base match
            ps = pp.tile([128, NPTS], F32, tag="ps")
            for c in range(4):
                sl = slice(c * NQ, (c + 1) * NQ)
                mm(ps[:, sl], wsm_t[64:128, 448:576], bcl[64:128, sl],
                   start=True, stop=True)
                if c % 2 == 0:
                    drain_act(ps[:, sl], cg[:, sl], bias("cg", 128))
                else:
                    drain_dve_relu(ps[:, sl], cg[:, sl], bias("cg", 128))

            # ---- t64 relation: (128 -> 512 relu -> 64) on tpe ----
            h64 = hp.tile([128, 4, NPTS], F32R, tag="hbig")
            for m in range(4):
                ps = pp.tile([128, NPTS], F32, tag="ps")
                for n in range(2):
                    sl = slice(n * NHALF, (n + 1) * NHALF)
                    mm(ps[:, sl], W["wt64a"][:, m * 128:(m + 1) * 128],
                       tpe[:, sl], start=True, stop=True)
                drain2(ps[:], h64[:, m, :], bias("t64a", 128, m))

            # p duplicated into both halves of a [128, n] tile, then the
            # residual add; chunked so the add pipelines behind the drain
            psb = ap_.tile([128, NPTS], F32, tag="psb")
            ps = pp.tile([64, NPTS], F32, tag="ps")
            for n in range(2):
                sl = slice(n * NHALF, (n + 1) * NHALF)
                for k in range(4):
                    mm(ps[:, sl], W["wt64b"][:, k * 64:(k + 1) * 64],
                       h64[:, k, sl], start=(k == 0), stop=(k == 3))
            for c in range(4):
                sl = slice(c * NQ, (c + 1) * NQ)
                drain_dve_bias(ps[:, sl], psb[0:64, sl], bias("t64b", 64))
                # shifted write (rows 0:64 -> 64:128): ACT only
                drain_act(ps[:, sl], psb[64:128, sl], bias("t64b", 64),
                          relu=False)
                nc.vector.tensor_add(tpe[:, sl].bitcast(F32R),
                                     tpe[:, sl].bitcast(F32), psb[:, sl])

            # ---- inst_global ----
            ps = pp.tile([128, NPTS], F32, tag="ps")
            for n in range(2):
                sl = slice(n * NHALF, (n + 1) * NHALF)
                mm(ps[:, sl], wsm_t[:, 576:704], tpe[:, sl], start=True, stop=True)
            drain2(ps[:], ig[:], bias("iglob", 128))

            # ---- t128 relation: (256 -> 512 relu -> 128) on [ig; cg] ----
            h128 = hp.tile([128, 4, NPTS], F32R, tag="hbig")
            for m in range(4):
                ps = pp.tile([128, NPTS], F32, tag="ps")
                for n in range(2):
                    sl = slice(n * NHALF, (n + 1) * NHALF)
                    for k, src in ((0, ig), (1, cg)):
                        mm(ps[:, sl],
                           W["wt128a"][:, k * 512 + m * 128:k * 512 + (m + 1) * 128],
                           src[:, sl], start=(k == 0), stop=(k == 1))
                drain2(ps[:], h128[:, m, :], bias("t128a", 128, m))

            qsb = ap_.tile([128, NPTS], F32, tag="qsb")
            ps = pp.tile([128, NPTS], F32, tag="ps")
            for n in range(2):
                sl = slice(n * NHALF, (n + 1) * NHALF)
                for k in range(4):
                    mm(ps[:, sl], W["wt128b"][:, k * 128:(k + 1) * 128],
                       h128[:, k, sl], start=(k == 0), stop=(k == 3))
            # chunked drain + ig add (DVE) pipelined behind ACT; the cg add
            # runs on GpSimd in parallel (only gates de0)
            for c in range(4):
                sl = slice(c * NQ, (c + 1) * NQ)
                drain_act(ps[:, sl], qsb[:, sl], bias("t128b", 128), relu=False)
                nc.vector.tensor_add(ig[:, sl].bitcast(F32R),
                                     ig[:, sl].bitcast(F32), qsb[:, sl])
            nc.gpsimd.tensor_add(cg[:].bitcast(F32R), cg[:].bitcast(F32), qsb[:])

            # ---- assign & deform heads ----
            ah1 = hp.tile([128, 4, NPTS], F32R, tag="hbig")
            dh1 = hp.tile([128, 4, NPTS], F32R, tag="hbig")
            ah2 = hp.tile([128, 2, NPTS], F32R, tag="h2")
            dh2 = hp.tile([128, 2, NPTS], F32R, tag="h2")

            def head_l1_tile(wkey, src, dst, bkey, m):
                ps = pp.tile([128, NPTS], F32, tag="ps")
                for n in range(2):
                    sl = slice(n * NHALF, (n + 1) * NHALF)
                    mm(ps[:, sl], W[wkey][:, m * 128:(m + 1) * 128],
                       src[:, sl], start=True, stop=True)
                drain2(ps[:], dst[:, m, :], bias(bkey, 128, m))

            def head_l2_tile(wkey, src, dst, bkey, m):
                ps = pp.tile([128, NPTS], F32, tag="ps")
                for n in range(2):
                    sl = slice(n * NHALF, (n + 1) * NHALF)
                    for k in range(4):
                        mm(ps[:, sl],
                           W[wkey][:, k * 256 + m * 128:k * 256 + (m + 1) * 128],
                           src[:, k, sl], start=(k == 0), stop=(k == 3))
                drain2(ps[:], dst[:, m, :], bias(bkey, 128, m))

            # assign path first (as0 -> as1 -> as2) so the big output DMAs
            # start as early as possible; the deform head interleaves into
            # the as2 phase so the PE keeps computing while 4 MB of output
            # streams to HBM
            for m in range(4):
                head_l1_tile("was0", ig, ah1, "as0", m)
            for m in range(2):
                head_l2_tile("was1", ah1, ah2, "as1", m)

            def as2_tile(m):
                ps = pp.tile([128, NPTS], F32, tag="ps")
                for n in range(2):
                    sl = slice(n * NHALF, (n + 1) * NHALF)
                    for k in range(2):
                        mm(ps[:, sl],
                           W["was2"][:, k * 1024 + m * 128:k * 1024 + (m + 1) * 128],
                           ah2[:, k, sl], start=(k == 0), stop=(k == 1))
                ot = op_.tile([128, NPTS], F32, tag="oa")
                drain2(ps[:], ot[:], bias("as2", 128, m), relu=False)
                nc.sync.dma_start(out_a[m * 128:(m + 1) * 128, :], ot[:])

            head_l1_tile("wde0", cg, dh1, "de0", 0)
            for m in range(3):
                as2_tile(m)
                head_l1_tile("wde0", cg, dh1, "de0", m + 1)
            as2_tile(3)
            head_l2_tile("wde1", dh1, dh2, "de1", 0)
            as2_tile(4)
            head_l2_tile("wde1", dh1, dh2, "de1", 1)
            as2_tile(5)
            as2_tile(6)

            ps = pp.tile([3, NPTS], F32, tag="ps")
            for n in range(2):
                sl = slice(n * NHALF, (n + 1) * NHALF)
                for k in range(2):
                    mm(ps[:, sl], W["wde2"][:, k * 3:(k + 1) * 3],
                       dh2[:, k, sl], start=(k == 0), stop=(k == 1))
            od = op_.tile([3, NPTS], F32, tag="od")
            drain_dve_bias(ps[:], od[:], bias("de2", 3))
            nc.sync.dma_start(out_d[:], od[:])

            as2_tile(7)

    nc.compile()
    return nc


def _get_program():
    global _PROGRAM
    if _PROGRAM is None:
        _PROGRAM = _build_program()
    return _PROGRAM


def _pack_blocks(wt, block_cols):
    """[K, M] with K = nk*128 -> [128, nk*M] (K-tile blocks side by side)."""
    K, M = wt.shape
    nk = K // 128
    assert nk * 128 == K and M == block_cols
    return np.concatenate([wt[i * 128:(i + 1) * 128, :] for i in range(nk)], axis=1)


def _np(x, dtype=None):
    try:
        a = np.asarray(x)
    except Exception:
        import jax
        a = np.asarray(jax.device_get(x))
    return a.astype(dtype) if dtype is not None and a.dtype != dtype else a


def _host_pack(points, emb_map, choose, cat_id, prior, params):
    """Build the 8 per-core input maps."""
    p = {k: [(_np(w, np.float32), _np(b, np.float32)) for w, b in v]
         for k, v in params.items()}

    def wT(key, i):
        return np.ascontiguousarray(p[key][i][0].T)

    wsm = np.zeros((128, WSM_W), np.float32)
    # block-diagonal pairs: rows 0:K_ig cols 0:64 = ig_i, rows K.. cols 64:128 = cl_i
    wsm[0:3, 0:64] = wT("ig", 0)
    wsm[3:6, 64:128] = wT("cl", 0)
    wsm[0:64, 128:192] = wT("ig", 1)
    wsm[64:128, 192:256] = wT("cl", 1)
    wsm[0:64, 256:320] = wT("ig", 2)
    wsm[64:128, 320:384] = wT("cl", 2)
    wsm[0:32, 384:448] = wT("ic", 0)
    wsm[64:128, 448:576] = wT("cg", 0)   # rows 64:128: rhs lives there too
    wsm[0:128, 576:704] = wT("iglob", 0)

    def put_bias(vec, col, prow=0):
        vec = np.asarray(vec, np.float32).reshape(-1)
        wsm[prow:prow + vec.size, BIAS_BASE + col] = vec

    put_bias(np.concatenate([p["ig"][0][1], p["cl"][0][1]]), BIAS_COLS["p1"])
    put_bias(np.concatenate([p["ig"][1][1], p["cl"][1][1]]), BIAS_COLS["p2"])
    put_bias(np.concatenate([p["ig"][2][1], p["cl"][2][1]]), BIAS_COLS["p3"])
    put_bias(p["ic"][0][1], BIAS_COLS["ic"])
    put_bias(p["cg"][0][1], BIAS_COLS["cg"])
    put_bias(p["iglob"][0][1], BIAS_COLS["iglob"])

    def put_bias_tiles(key, i, col):
        b = p[key][i][1]
        n = b.size // 128
        for j in range(n):
            put_bias(b[j * 128:(j + 1) * 128], col + j)

    put_bias_tiles("t64", 0, BIAS_COLS["t64a"])
    put_bias(p["t64"][1][1], BIAS_COLS["t64b"])
    put_bias_tiles("t128", 0, BIAS_COLS["t128a"])
    put_bias(p["t128"][1][1], BIAS_COLS["t128b"])
    put_bias_tiles("assign", 0, BIAS_COLS["as0"])
    put_bias_tiles("assign", 1, BIAS_COLS["as1"])
    put_bias_tiles("deform", 0, BIAS_COLS["de0"])
    put_bias_tiles("deform", 1, BIAS_COLS["de1"])

    base = {
        "wt64a": wT("t64", 0),
        "wt64b": _pack_blocks(wT("t64", 1), 64),
        "wt128a": _pack_blocks(wT("t128", 0), 512),
        "wt128b": _pack_blocks(wT("t128", 1), 128),
        "was0": wT("assign", 0),
        "was1": _pack_blocks(wT("assign", 1), 256),
        "wde0": wT("deform", 0),
        "wde1": _pack_blocks(wT("deform", 1), 256),
    }

    points = _np(points, np.float32)
    prior = _np(prior, np.float32)
    emb_map = _np(emb_map, np.float32)
    choose = _np(choose, np.int64)
    cat_id = _np(cat_id, np.int64)

    was2_w = p["assign"][2][0]   # (6144, 256)
    was2_b = p["assign"][2][1]   # (6144,)
    wde2_w = p["deform"][2][0]   # (18, 256)
    wde2_b = p["deform"][2][1]   # (18,)

    in_maps = []
    for i in range(8):
        cat = int(cat_id[i])
        wsmi = wsm.copy()
        b8 = was2_b[cat * NV:(cat + 1) * NV].reshape(8, 128)
        for j in range(8):
            wsmi[0:128, BIAS_BASE + BIAS_COLS["as2"] + j] = b8[j]
        wsmi[0:3, BIAS_BASE + BIAS_COLS["de2"]] = wde2_b[cat * 3:cat * 3 + 3]

        xin = np.empty((38, NPTS), np.float32)
        xin[0:3] = points[i].T
        xin[3:6] = prior[i].T
        xin[6:38] = emb_map[i][:, choose[i]]

        m = dict(base)
        m["wsm"] = wsmi
        m["xin"] = xin
        m["was2"] = _pack_blocks(
            np.ascontiguousarray(was2_w[cat * NV:(cat + 1) * NV, :].T), NV)
        m["wde2"] = _pack_blocks(
            np.ascontiguousarray(wde2_w[cat * 3:cat * 3 + 3, :].T), 3)
        in_maps.append(m)
    return in_maps


# Optional override used by test.py to run with NTFF profiling; the graded
# path never sets this.
RUNNER = None


def _assemble(results):
    assign = np.stack([results[i]["assign_T"].T for i in range(8)])
    deltas = np.stack([results[i]["deltas_T"].T for i in range(8)])
    return np.ascontiguousarray(assign), np.ascontiguousarray(deltas)


def kernel(points, emb_map, choose, cat_id, prior, params):
    nc = _get_program()
    in_maps = _host_pack(points, emb_map, choose, cat_id, prior, params)
    if RUNNER is not None:
        results = RUNNER(nc, in_maps)
    else:
        results = run_bass_kernel_spmd(nc, in_maps, list(range(8))).results
    return _assemble(results)


# revision 29
# speedup vs baseline: 1.1529x; 1.1529x over previous
"""Trainium2 Bass kernel for nn_DeformNet (dense per-point MLP network).

Strategy: pure data parallel over batch (bs=8 -> 1 batch item per NeuronCore).
All matmuls run in "channels on partitions" layout: activations are [C, n_pts]
tiles, weights are pre-transposed [Cin, Cout] (lhsT), so out = W @ x lands as
[Cout, n_pts] in PSUM. The per-sample category selection (cat_id) is applied
host-side by slicing the final assignment/deformation weight rows before
shipping them to the device (each core only computes the 1024 assignment
channels + 3 deformation channels its batch item actually needs). The big
[nv, n_pts] result is produced transposed on device and swapped on the host.

Matmuls use the float32r PE mode (TF32-like rounding, ~234 ns/matmul at
N=512 when warm). The instance-geometry and category-local 3-layer chains
are fused pairwise with block-diagonal weights so one matmul computes both.
PSUM drains are split between ScalarE and VectorE; a few fp32 dummy matmuls
at kernel start pre-warm the PE clock (HAM) before the real stream begins.
"""

import os
import sys

for _p in ("/opt/trn_rl_repo", "/root/.axon_site/_ro/trn_rl_repo"):
    if _p not in sys.path and os.path.isdir(_p):
        sys.path.append(_p)

import numpy as np

import concourse.bass as bass
import concourse.tile as tile
from concourse import bacc, mybir
from concourse.bass_utils import run_bass_kernel_spmd

F32 = mybir.dt.float32
F32R = mybir.dt.float32r
RELU = mybir.ActivationFunctionType.Relu
COPY = mybir.ActivationFunctionType.Identity
ADD = mybir.AluOpType.add
MAX = mybir.AluOpType.max

NPTS = 1024
NV = 1024
NCAT = 6
NHALF = 512  # fp32 moving-operand max per matmul

WSM_W = 1000  # 704 weight cols + 37 bias cols + bias-row regions
BIAS_BASE = 704

# bias column map (columns BIAS_BASE+c of the wsm tile)
BIAS_COLS = {
    "p1": 0, "p2": 1, "p3": 2,   # paired ig/cl biases: rows 0:64 ig, 64:128 cl
    "ic": 3, "cg": 4, "iglob": 5,
    "t64a": 6,    # 4 cols
    "t64b": 10,
    "t128a": 11,  # 4 cols
    "t128b": 15,
    "as0": 16,    # 4 cols
    "as1": 20,    # 2 cols
    "as2": 22,    # 8 cols
    "de0": 30,    # 4 cols
    "de1": 34,    # 2 cols
    "de2": 36,
}
# row-0 bias rows (for K=1 bias matmuls): [b_t64b, b_t64b] then b_t128b
T64B_BROW = 744
T128B_BROW = 872

_PROGRAM = None


def _build_program():
    nc = bacc.Bacc("TRN2", target_bir_lowering=False, debug=False, num_devices=8)

    dram_in = {}
    for name, shape in [
        ("xin", [39, NPTS]),
        ("wsm", [128, WSM_W]),
        ("wt64a", [128, 512]), ("wt64b", [128, 512]),
        ("wt128a", [128, 1024]), ("wt128b", [128, 512]),
        ("was0", [128, 512]), ("was1", [128, 1024]), ("was2", [128, 2048]),
        ("wde0", [128, 512]), ("wde1", [128, 1024]), ("wde2", [128, 6]),
    ]:
        dram_in[name] = nc.dram_tensor(name, shape, F32, kind="ExternalInput")

    out_a = nc.dram_tensor("assign_T", [NV, NPTS], F32, kind="ExternalOutput")
    out_d = nc.dram_tensor("deltas_T", [3, NPTS], F32, kind="ExternalOutput")

    with tile.TileContext(nc) as tc:
        with (
            tc.tile_pool(name="w", bufs=1) as wp,
            tc.tile_pool(name="a", bufs=1) as ap_,
            tc.tile_pool(name="hb", bufs=2) as hp,
            tc.tile_pool(name="ps", bufs=4, space="PSUM") as pp,
            tc.tile_pool(name="o", bufs=4) as op_,
        ):
            # ---- critical-path loads first on sync; big weights spread
            # across the other DMA-capable sequencers ----
            wsm_t = wp.tile([128, WSM_W], F32R, tag="wsm")
            nc.sync.dma_start(wsm_t[:], dram_in["wsm"][:].bitcast(F32R))
            xpp = wp.tile([6, NPTS], F32R, tag="xpp")
            xemb = wp.tile([32, NPTS], F32R, tag="xemb")
            xones = wp.tile([1, NPTS], F32R, tag="xones")
            nc.sync.dma_start(xpp[:], dram_in["xin"][0:6, :].bitcast(F32R))
            nc.sync.dma_start(xemb[:], dram_in["xin"][6:38, :].bitcast(F32R))
            nc.sync.dma_start(xones[:], dram_in["xin"][38:39, :].bitcast(F32R))

            # ---- PE warm-up: fp32 dummy matmuls (4 cyc/row -> dense PE
            # busy) prime the HAM activity window so the real matmul stream
            # runs at 2.4 GHz ----
            wz = wp.tile([128, NHALF], F32, tag="wz")
            nc.vector.memset(wz[:], 0.0)
            for i in range(5):
                psw = pp.tile([128, NHALF], F32, tag="ps")
                nc.tensor.matmul(psw[:], wz[:, 0:128], wz[:],
                                 start=True, stop=True)

            DMA_ENG = {
                "wt64a": nc.scalar, "wt64b": nc.scalar,
                "wt128a": nc.scalar, "wt128b": nc.scalar,
                "was0": nc.gpsimd, "wde0": nc.gpsimd,
                "was1": nc.gpsimd, "wde1": nc.gpsimd,
                "was2": nc.sync, "wde2": nc.sync,
            }
            W = {"wsm": wsm_t}
            for name in ("wt64a", "wt64b", "wt128a", "wt128b",
                         "was0", "wde0", "was1", "wde1", "was2", "wde2"):
                cols = dram_in[name].shape[1]
                t = wp.tile([128, cols], F32R, tag=name)
                DMA_ENG[name].dma_start(t[:], dram_in[name][:].bitcast(F32R))
                W[name] = t

            wsmb = wsm_t[:].bitcast(F32)

            def bias(key, rows, off=0, prow=0):
                c = BIAS_BASE + BIAS_COLS[key] + off
                return wsmb[prow:prow + rows, c:c + 1]

            def mm(ps, lhsT, rhs, start, stop):
                nc.tensor.matmul(ps, lhsT, rhs, start=start, stop=stop)

            def drain_act(ps, dst, b, relu=True):
                nc.scalar.activation(dst, ps, RELU if relu else COPY, bias=b)

            def drain_dve_relu(ps, dst, b):
                nc.vector.tensor_scalar(dst, ps, b, 0.0, ADD, MAX)

            def drain_dve_bias(ps, dst, b):
                nc.vector.tensor_scalar(dst, ps, b, None, ADD)

            def drain2(ps, dst, b, relu=True):
                # split the drain: ACT takes the first half, DVE the second,
                # so PSUM evacuation latency halves and both engines load
                h = NHALF
                drain_act(ps[:, 0:h], dst[:, 0:h], b, relu)
                if relu:
                    drain_dve_relu(ps[:, h:2 * h], dst[:, h:2 * h], b)
                else:
                    drain_dve_bias(ps[:, h:2 * h], dst[:, h:2 * h], b)

            # ---- fused instance-geometry (rows 0:64) + category-local
            # (rows 64:128) chains via block-diagonal weights.  This serial
            # chain is latency-bound, so it runs at 256-wide chunks with
            # the drains pipelined behind the matmuls ----
            ab1 = ap_.tile([128, NPTS], F32R, tag="ab1")
            ab2 = ap_.tile([128, NPTS], F32R, tag="ab2")
            bcl = ap_.tile([128, NPTS], F32R, tag="bcl")  # cl3 out in rows 64:128
            tpe = ap_.tile([128, NPTS], F32R, tag="tpe")
            ig = ap_.tile([128, NPTS], F32R, tag="ig")
            cg = ap_.tile([128, NPTS], F32R, tag="cg")

            NQ = 256

            # pair1: K=6 rows (pts rows 0:3, prior rows 3:6)
            ps = pp.tile([128, NPTS], F32, tag="ps")
            for c in range(4):
                sl = slice(c * NQ, (c + 1) * NQ)
                mm(ps[:, sl], wsm_t[0:6, 0:128], xpp[0:6, sl],
                   start=True, stop=True)
                if c % 2 == 0:
                    drain_act(ps[:, sl], ab1[:, sl], bias("p1", 128))
                else:
                    drain_dve_relu(ps[:, sl], ab1[:, sl], bias("p1", 128))

            # ic is independent -> fills the PE while ab1 drains; its drain
            # is a partition-shifted write (psum rows 0:64 -> tpe rows
            # 64:128), which only ACT handles
            ps_ic = pp.tile([64, NPTS], F32, tag="ps")
            for n in range(2):
                sl = slice(n * NHALF, (n + 1) * NHALF)
                mm(ps_ic[:, sl], wsm_t[0:32, 384:448], xemb[:, sl],
                   start=True, stop=True)
                drain_act(ps_ic[:, sl], tpe[64:128, sl], bias("ic", 64))

            # pair2
            ps = pp.tile([128, NPTS], F32, tag="ps")
            for c in range(4):
                sl = slice(c * NQ, (c + 1) * NQ)
                mm(ps[:, sl], wsm_t[0:128, 128:256], ab1[:, sl],
                   start=True, stop=True)
                if c % 2 == 0:
                    drain_act(ps[:, sl], ab2[:, sl], bias("p2", 128))
                else:
                    drain_dve_relu(ps[:, sl], ab2[:, sl], bias("p2", 128))

            # pair3: rows 0:64 -> tpe (pts path), rows 64:128 -> bcl (cl3)
            ps = pp.tile([128, NPTS], F32, tag="ps")
            for c in range(4):
                sl = slice(c * NQ, (c + 1) * NQ)
                mm(ps[:, sl], wsm_t[0:128, 256:384], ab2[:, sl],
                   start=True, stop=True)
                drain_act(ps[0:64, sl], tpe[0:64, sl], bias("p3", 64))
                drain_dve_relu(ps[64:128, sl], bcl[64:128, sl],
                               bias("p3", 64, prow=64))

            # cg: weights placed at wsm rows 64:128 so lhsT/rhs base match
            ps = pp.tile([128, NPTS], F32, tag="ps")
            for c in range(4):
                sl = slice(c * NQ, (c + 1) * NQ)
                mm(ps[:, sl], wsm_t[64:128, 448:576], bcl[64:128, sl],
                   start=True, stop=True)
                if c % 2 == 0:
                    drain_act(ps[:, sl], cg[:, sl], bias("cg", 128))
                else:
                    drain_dve_relu(ps[:, sl], cg[:, sl], bias("cg", 128))

            # ---- t64 relation: (128 -> 512 relu -> 64) on tpe ----
            h64 = hp.tile([128, 4, NPTS], F32R, tag="hbig")
            for m in range(4):
                ps = pp.tile([128, NPTS], F32, tag="ps")
                for n in range(2):
                    sl = slice(n * NHALF, (n + 1) * NHALF)
                    mm(ps[:, sl], W["wt64a"][:, m * 128:(m + 1) * 128],
                       tpe[:, sl], start=True, stop=True)
                drain2(ps[:], h64[:, m, :], bias("t64a", 128, m))

            # t64b: the weight columns are duplicated host-side so p lands
            # in BOTH psum row-halves (M=128, full array), and the bias is
            # accumulated by a K=1 ones-row matmul -- the residual add then
            # reads PSUM directly, no staging tile
            ps = pp.tile([128, NPTS], F32, tag="ps")
            for n in range(2):
                sl = slice(n * NHALF, (n + 1) * NHALF)
                mm(ps[:, sl], wsm_t[0:1, T64B_BROW:T64B_BROW + 128],
                   xones[:, sl], start=True, stop=False)
                for k in range(4):
                    mm(ps[:, sl], W["wt64b"][:, k * 128:(k + 1) * 128],
                       h64[:, k, sl], start=False, stop=(k == 3))
                nc.vector.tensor_add(tpe[:, sl].bitcast(F32R),
                                     tpe[:, sl].bitcast(F32), ps[:, sl])

            # ---- inst_global ----
            ps = pp.tile([128, NPTS], F32, tag="ps")
            for n in range(2):
                sl = slice(n * NHALF, (n + 1) * NHALF)
                mm(ps[:, sl], wsm_t[:, 576:704], tpe[:, sl], start=True, stop=True)
            drain2(ps[:], ig[:], bias("iglob", 128))

            # ---- t128 relation: (256 -> 512 relu -> 128) on [ig; cg] ----
            h128 = hp.tile([128, 4, NPTS], F32R, tag="hbig")
            for m in range(4):
                ps = pp.tile([128, NPTS], F32, tag="ps")
                for n in range(2):
                    sl = slice(n * NHALF, (n + 1) * NHALF)
                    for k, src in ((0, ig), (1, cg)):
                        mm(ps[:, sl],
                           W["wt128a"][:, k * 512 + m * 128:k * 512 + (m + 1) * 128],
                           src[:, sl], start=(k == 0), stop=(k == 1))
                drain2(ps[:], h128[:, m, :], bias("t128a", 128, m))

            qsb = ap_.tile([128, NPTS], F32, tag="qsb")
            ps = pp.tile([128, NPTS], F32, tag="ps")
            for n in range(2):
                sl = slice(n * NHALF, (n + 1) * NHALF)
                mm(ps[:, sl], wsm_t[0:1, T128B_BROW:T128B_BROW + 128],
                   xones[:, sl], start=True, stop=False)
                for k in range(4):
                    mm(ps[:, sl], W["wt128b"][:, k * 128:(k + 1) * 128],
                       h128[:, k, sl], start=False, stop=(k == 3))
                # ig add straight from PSUM (critical path to as0); ACT
                # copies q aside for the cg add on GpSimd (only gates de0)
                nc.vector.tensor_add(ig[:, sl].bitcast(F32R),
                                     ig[:, sl].bitcast(F32), ps[:, sl])
                drain_act(ps[:, sl], qsb[:, sl], 0.0, relu=False)
            nc.gpsimd.tensor_add(cg[:].bitcast(F32R), cg[:].bitcast(F32), qsb[:])

            # ---- assign & deform heads ----
            ah1 = hp.tile([128, 4, NPTS], F32R, tag="hbig")
            dh1 = hp.tile([128, 4, NPTS], F32R, tag="hbig")
            ah2 = hp.tile([128, 2, NPTS], F32R, tag="h2")
            dh2 = hp.tile([128, 2, NPTS], F32R, tag="h2")

            def head_l1_tile(wkey, src, dst, bkey, m):
                ps = pp.tile([128, NPTS], F32, tag="ps")
                for n in range(2):
                    sl = slice(n * NHALF, (n + 1) * NHALF)
                    mm(ps[:, sl], W[wkey][:, m * 128:(m + 1) * 128],
                       src[:, sl], start=True, stop=True)
                drain2(ps[:], dst[:, m, :], bias(bkey, 128, m))

            def head_l2_tile(wkey, src, dst, bkey, m):
                ps = pp.tile([128, NPTS], F32, tag="ps")
                for n in range(2):
                    sl = slice(n * NHALF, (n + 1) * NHALF)
                    for k in range(4):
                        mm(ps[:, sl],
                           W[wkey][:, k * 256 + m * 128:k * 256 + (m + 1) * 128],
                           src[:, k, sl], start=(k == 0), stop=(k == 3))
                drain2(ps[:], dst[:, m, :], bias(bkey, 128, m))

            # assign path first (as0 -> as1 -> as2) so the big output DMAs
            # start as early as possible; the deform head interleaves into
            # the as2 phase so the PE keeps computing while 4 MB of output
            # streams to HBM
            for m in range(4):
                head_l1_tile("was0", ig, ah1, "as0", m)
            for m in range(2):
                head_l2_tile("was1", ah1, ah2, "as1", m)

            def as2_tile(m):
                ps = pp.tile([128, NPTS], F32, tag="ps")
                for n in range(2):
                    sl = slice(n * NHALF, (n + 1) * NHALF)
                    for k in range(2):
                        mm(ps[:, sl],
                           W["was2"][:, k * 1024 + m * 128:k * 1024 + (m + 1) * 128],
                           ah2[:, k, sl], start=(k == 0), stop=(k == 1))
                ot = op_.tile([128, NPTS], F32, tag="oa")
                drain2(ps[:], ot[:], bias("as2", 128, m), relu=False)
                nc.sync.dma_start(out_a[m * 128:(m + 1) * 128, :], ot[:])

            head_l1_tile("wde0", cg, dh1, "de0", 0)
            for m in range(3):
                as2_tile(m)
                head_l1_tile("wde0", cg, dh1, "de0", m + 1)
            as2_tile(3)
            head_l2_tile("wde1", dh1, dh2, "de1", 0)
            as2_tile(4)
            head_l2_tile("wde1", dh1, dh2, "de1", 1)
            as2_tile(5)
            as2_tile(6)

            ps = pp.tile([3, NPTS], F32, tag="ps")
            for n in range(2):
                sl = slice(n * NHALF, (n + 1) * NHALF)
                for k in range(2):
                    mm(ps[:, sl], W["wde2"][:, k * 3:(k + 1) * 3],
                       dh2[:, k, sl], start=(k == 0), stop=(k == 1))
            od = op_.tile([3, NPTS], F32, tag="od")
            drain_dve_bias(ps[:], od[:], bias("de2", 3))
            nc.sync.dma_start(out_d[:], od[:])

            as2_tile(7)

    nc.compile()
    return nc


def _get_program():
    global _PROGRAM
    if _PROGRAM is None:
        _PROGRAM = _build_program()
    return _PROGRAM


def _pack_blocks(wt, block_cols):
    """[K, M] with K = nk*128 -> [128, nk*M] (K-tile blocks side by side)."""
    K, M = wt.shape
    nk = K // 128
    assert nk * 128 == K and M == block_cols
    return np.concatenate([wt[i * 128:(i + 1) * 128, :] for i in range(nk)], axis=1)


def _np(x, dtype=None):
    try:
        a = np.asarray(x)
    except Exception:
        import jax
        a = np.asarray(jax.device_get(x))
    return a.astype(dtype) if dtype is not None and a.dtype != dtype else a


def _host_pack(points, emb_map, choose, cat_id, prior, params):
    """Build the 8 per-core input maps."""
    p = {k: [(_np(w, np.float32), _np(b, np.float32)) for w, b in v]
         for k, v in params.items()}

    def wT(key, i):
        return np.ascontiguousarray(p[key][i][0].T)

    wsm = np.zeros((128, WSM_W), np.float32)
    # block-diagonal pairs: rows 0:K_ig cols 0:64 = ig_i, rows K.. cols 64:128 = cl_i
    wsm[0:3, 0:64] = wT("ig", 0)
    wsm[3:6, 64:128] = wT("cl", 0)
    wsm[0:64, 128:192] = wT("ig", 1)
    wsm[64:128, 192:256] = wT("cl", 1)
    wsm[0:64, 256:320] = wT("ig", 2)
    wsm[64:128, 320:384] = wT("cl", 2)
    wsm[0:32, 384:448] = wT("ic", 0)
    wsm[64:128, 448:576] = wT("cg", 0)   # rows 64:128: rhs lives there too
    wsm[0:128, 576:704] = wT("iglob", 0)

    def put_bias(vec, col, prow=0):
        vec = np.asarray(vec, np.float32).reshape(-1)
        wsm[prow:prow + vec.size, BIAS_BASE + col] = vec

    put_bias(np.concatenate([p["ig"][0][1], p["cl"][0][1]]), BIAS_COLS["p1"])
    put_bias(np.concatenate([p["ig"][1][1], p["cl"][1][1]]), BIAS_COLS["p2"])
    put_bias(np.concatenate([p["ig"][2][1], p["cl"][2][1]]), BIAS_COLS["p3"])
    put_bias(p["ic"][0][1], BIAS_COLS["ic"])
    put_bias(p["cg"][0][1], BIAS_COLS["cg"])
    put_bias(p["iglob"][0][1], BIAS_COLS["iglob"])

    def put_bias_tiles(key, i, col):
        b = p[key][i][1]
        n = b.size // 128
        for j in range(n):
            put_bias(b[j * 128:(j + 1) * 128], col + j)

    put_bias_tiles("t64", 0, BIAS_COLS["t64a"])
    put_bias_tiles("t128", 0, BIAS_COLS["t128a"])
    # bias ROWS (row 0) for the K=1 bias matmuls
    b64 = p["t64"][1][1].reshape(-1)
    wsm[0, T64B_BROW:T64B_BROW + 128] = np.concatenate([b64, b64])
    wsm[0, T128B_BROW:T128B_BROW + 128] = p["t128"][1][1].reshape(-1)
    put_bias_tiles("assign", 0, BIAS_COLS["as0"])
    put_bias_tiles("assign", 1, BIAS_COLS["as1"])
    put_bias_tiles("deform", 0, BIAS_COLS["de0"])
    put_bias_tiles("deform", 1, BIAS_COLS["de1"])

    wt64b_T = wT("t64", 1)  # (512, 64)
    wt64b_dup = np.concatenate([wt64b_T, wt64b_T], axis=1)  # (512, 128)
    base = {
        "wt64a": wT("t64", 0),
        "wt64b": _pack_blocks(wt64b_dup, 128),
        "wt128a": _pack_blocks(wT("t128", 0), 512),
        "wt128b": _pack_blocks(wT("t128", 1), 128),
        "was0": wT("assign", 0),
        "was1": _pack_blocks(wT("assign", 1), 256),
        "wde0": wT("deform", 0),
        "wde1": _pack_blocks(wT("deform", 1), 256),
    }

    points = _np(points, np.float32)
    prior = _np(prior, np.float32)
    emb_map = _np(emb_map, np.float32)
    choose = _np(choose, np.int64)
    cat_id = _np(cat_id, np.int64)

    was2_w = p["assign"][2][0]   # (6144, 256)
    was2_b = p["assign"][2][1]   # (6144,)
    wde2_w = p["deform"][2][0]   # (18, 256)
    wde2_b = p["deform"][2][1]   # (18,)

    in_maps = []
    for i in range(8):
        cat = int(cat_id[i])
        wsmi = wsm.copy()
        b8 = was2_b[cat * NV:(cat + 1) * NV].reshape(8, 128)
        for j in range(8):
            wsmi[0:128, BIAS_BASE + BIAS_COLS["as2"] + j] = b8[j]
        wsmi[0:3, BIAS_BASE + BIAS_COLS["de2"]] = wde2_b[cat * 3:cat * 3 + 3]

        xin = np.empty((39, NPTS), np.float32)
        xin[0:3] = points[i].T
        xin[3:6] = prior[i].T
        xin[6:38] = emb_map[i][:, choose[i]]
        xin[38] = 1.0

        m = dict(base)
        m["wsm"] = wsmi
        m["xin"] = xin
        m["was2"] = _pack_blocks(
            np.ascontiguousarray(was2_w[cat * NV:(cat + 1) * NV, :].T), NV)
        m["wde2"] = _pack_blocks(
            np.ascontiguousarray(wde2_w[cat * 3:cat * 3 + 3, :].T), 3)
        in_maps.append(m)
    return in_maps


# Optional override used by test.py to run with NTFF profiling; the graded
# path never sets this.
RUNNER = None


def _assemble(results):
    assign = np.stack([results[i]["assign_T"].T for i in range(8)])
    deltas = np.stack([results[i]["deltas_T"].T for i in range(8)])
    return np.ascontiguousarray(assign), np.ascontiguousarray(deltas)


def kernel(points, emb_map, choose, cat_id, prior, params):
    nc = _get_program()
    in_maps = _host_pack(points, emb_map, choose, cat_id, prior, params)
    if RUNNER is not None:
        results = RUNNER(nc, in_maps)
    else:
        results = run_bass_kernel_spmd(nc, in_maps, list(range(8))).results
    return _assemble(results)
